# revision 3
# baseline (speedup 1.0000x reference)
import sys
import numpy as np

sys.path.insert(0, "/opt/trn_rl_repo")

import concourse.bass as bass  # noqa: E402
import concourse.tile as tile  # noqa: E402
import concourse.mybir as mybir  # noqa: E402
from concourse import bacc, bass_utils  # noqa: E402
from contextlib import ExitStack  # noqa: E402

F32 = mybir.dt.float32
BF16 = mybir.dt.bfloat16
I16 = mybir.dt.int16

B = 2048
IN = 2048
F = 2048
SIX = 6
LUT = 64
NCORES = 8
BLOC = B // NCORES      # 256 rows per core
W = F * SIX             # 12288 gather slots per layer
HALF = W // 2           # 6144

_CACHED = {}
TRACE = False
LAST = {}


def _brev6(k):
    r = 0
    for i in range(6):
        r |= ((k >> i) & 1) << (5 - i)
    return r


def _build_kernel():
    nc = bacc.Bacc("TRN2", debug=False)

    x0_d = nc.dram_tensor("x0", [BLOC, IN], F32, kind="ExternalInput").ap()
    r_ds = [nc.dram_tensor(f"r{l}", [BLOC, W], F32, kind="ExternalInput").ap()
            for l in range(3)]
    lut_ds = [nc.dram_tensor(f"lutT{l}", [LUT, F], F32, kind="ExternalInput").ap()
              for l in range(3)]
    idx_ds = [nc.dram_tensor(f"idxw{l}", [128, W // 16], I16, kind="ExternalInput").ap()
              for l in range(3)]
    pat_d = nc.dram_tensor("pat", [128, W], BF16, kind="ExternalInput").ap()
    out_d = nc.dram_tensor("out", [BLOC, F], F32, kind="ExternalOutput").ap()

    with tile.TileContext(nc) as tc:
        with ExitStack() as ctx:
            cpool = ctx.enter_context(tc.tile_pool(name="const", bufs=1))
            io = ctx.enter_context(tc.tile_pool(name="io", bufs=1))
            wk = ctx.enter_context(tc.tile_pool(name="wk", bufs=1))
            ps = ctx.enter_context(tc.tile_pool(name="ps", bufs=2, space="PSUM"))
            dr = ctx.enter_context(tc.tile_pool(name="dr", bufs=1, space="DRAM"))

            # constants resident in SBUF
            idxws = [cpool.tile([128, W // 16], I16, name=f"idxw{l}") for l in range(3)]
            for l in range(3):
                nc.sync.dma_start(idxws[l][:], idx_ds[l][:])
            pat = cpool.tile([128, W], BF16, name="pat")
            nc.sync.dma_start(pat[:], pat_d[:])
            ones1 = cpool.tile([1, 128], F32, name="ones1")
            nc.vector.memset(ones1[:], 1.0)
            halfb = cpool.tile([128, 1], F32, name="halfb")
            nc.vector.memset(halfb[:], 0.5)
            zerob = cpool.tile([128, 1], F32, name="zerob")
            nc.vector.memset(zerob[:], 0.0)

            # inter-layer activations in DRAM scratch
            xscr = dr.tile([2, BLOC, F], F32, name="xscr")

            for L in range(3):
                for bt in range(2):
                    xs = wk.tile([128, IN], F32, tag="xs", name="xs")
                    if L == 0:
                        xt = wk.tile([128, IN], F32, tag="xt", name="xt")
                        nc.sync.dma_start(xt[:], x0_d[bt * 128:(bt + 1) * 128, :])
                        # x = 0.5*in + 0.5
                        nc.scalar.activation(
                            xs[:], xt[:], mybir.ActivationFunctionType.Identity,
                            bias=halfb[:], scale=0.5)
                    else:
                        nc.sync.dma_start(
                            xs[:], xscr[(L + 1) % 2, bt * 128:(bt + 1) * 128, :])

                    bits = wk.tile([128, W], BF16, tag="bits", name="bits")
                    for h in range(2):
                        xg = wk.tile([128, HALF], F32, tag="xg", name="xg")
                        nc.gpsimd.ap_gather(
                            xg[:], xs[:], idxws[L][:, h * (HALF // 16):(h + 1) * (HALF // 16)],
                            channels=128, num_elems=IN, d=1, num_idxs=HALF)
                        rt = wk.tile([128, HALF], F32, tag="rt", name="rt")
                        nc.sync.dma_start(
                            rt[:], r_ds[L][bt * 128:(bt + 1) * 128,
                                           h * HALF:(h + 1) * HALF])
                        nc.vector.tensor_tensor(
                            bits[:, h * HALF:(h + 1) * HALF], xg[:], rt[:],
                            mybir.AluOpType.is_ge)

                    sc = wk.tile([128, W], BF16, tag="sc", name="sc")
                    nc.vector.tensor_tensor_scan(
                        sc[:], pat[:], bits[:], 0.0,
                        mybir.AluOpType.mult, mybir.AluOpType.add)

                    idx = wk.tile([128, F], BF16, tag="idx", name="idx")
                    nc.vector.tensor_copy(idx[:], sc[:, 5::6])

                    acc = wk.tile([128, F], F32, tag="acc", name="acc")
                    for k in range(LUT):
                        lrow = wk.tile([1, F], F32, tag="lrow", name="lrow")
                        nc.sync.dma_start(lrow[:], lut_ds[L][k:k + 1, :])
                        lutb = ps.tile([128, F], F32, tag="lutb", name="lutb")
                        for q in range(4):
                            nc.tensor.matmul(lutb[:, q * 512:(q + 1) * 512],
                                             ones1[:], lrow[:, q * 512:(q + 1) * 512],
                                             start=True, stop=True)
                        mask = wk.tile([128, F], mybir.dt.uint8, tag="mask", name="mask")
                        nc.vector.tensor_scalar(
                            mask[:], idx[:], float(k), None, mybir.AluOpType.is_equal)
                        nc.vector.copy_predicated(acc[:], mask[:], lutb[:])

                    if L < 2:
                        xn = wk.tile([128, F], F32, tag="xn", name="xn")
                        nc.scalar.activation(
                            xn[:], acc[:], mybir.ActivationFunctionType.Sigmoid,
                            bias=zerob[:], scale=1.0)
                        nc.sync.dma_start(
                            xscr[L % 2, bt * 128:(bt + 1) * 128, :], xn[:])
                    else:
                        nc.sync.dma_start(out_d[bt * 128:(bt + 1) * 128, :], acc[:])

    nc.compile()
    return nc


def _prep_host(inputs, r1, r2, r3, lut1, lut2, lut3, connect_1, connect_2, connect_3):
    rs = [np.ascontiguousarray(r.reshape(B, W).astype(np.float32))
          for r in (r1, r2, r3)]
    # lutT with bit-reversed index (scan packs MSB-first from j ascending)
    brev = np.array([_brev6(k) for k in range(LUT)])
    lutTs = []
    for lut in (lut1, lut2, lut3):
        lt = np.ascontiguousarray(lut.astype(np.float32)[:, brev].T)  # [64, F]
        lutTs.append(lt)
    # wrapped int16 index tiles for ap_gather
    idxws = []
    for c in (connect_1, connect_2, connect_3):
        flat = np.ascontiguousarray(c.astype(np.int64).reshape(W)).astype(np.int16)
        wrapped = flat.reshape(W // 16, 16).T            # [16, W/16]
        idxws.append(np.ascontiguousarray(np.tile(wrapped, (8, 1))).astype(np.int16))
    pat = np.tile(np.array([0, 2, 2, 2, 2, 2], np.float32), F)[None, :].repeat(128, 0)
    import ml_dtypes
    pat = pat.astype(ml_dtypes.bfloat16)
    return rs, lutTs, idxws, pat


def kernel(inputs, r1, r2, r3, lut1, lut2, lut3, connect_1, connect_2, connect_3):
    inputs = np.asarray(inputs, np.float32)
    rs, lutTs, idxws, pat = _prep_host(
        inputs, r1, r2, r3, lut1, lut2, lut3, connect_1, connect_2, connect_3)

    if "nc" not in _CACHED:
        _CACHED["nc"] = _build_kernel()
    nc = _CACHED["nc"]

    in_maps = []
    for c in range(NCORES):
        sl = slice(c * BLOC, (c + 1) * BLOC)
        m = {"x0": np.ascontiguousarray(inputs[sl]),
             "pat": pat}
        for l in range(3):
            m[f"r{l}"] = np.ascontiguousarray(rs[l][sl])
            m[f"lutT{l}"] = lutTs[l]
            m[f"idxw{l}"] = idxws[l]
        in_maps.append(m)

    if TRACE:
        import tempfile
        tmpdir = tempfile.mkdtemp(prefix="bass_trace_")
        res = bass_utils.run_bass_kernel_spmd(
            nc, in_maps, core_ids=list(range(NCORES)), trace=True, tmpdir=tmpdir)
        LAST["exec_ns"] = res.exec_time_ns
        LAST["trace_dir"] = tmpdir
        LAST["res"] = res
    else:
        res = bass_utils.run_bass_kernel_spmd(nc, in_maps, core_ids=list(range(NCORES)))
    out = np.concatenate([res.results[c]["out"] for c in range(NCORES)], axis=0)
    return out.astype(np.float32)

